# revision 9
# baseline (speedup 1.0000x reference)
"""Causal self-attention (B=2, S=2048, E=1024, H=16, D=64) on 8 trn2 NeuronCores.

Sharding: tensor-parallel over heads — 2 heads per core. Each core computes
q^T,k^T for its 2 heads, V directly in [token, dim] layout via a separate
GEMM (no XBAR transposes), runs causal attention, and multiplies by its
128-row slice of W_proj, producing a partial [4096, 1024] output (bf16).
The host sums the 8 partials and adds b_proj_eff.

v2 changes vs v1 (214999 ns):
  - per-chunk interleave: qkv(n) -> vdir(n) -> attention(b,qc) -> proj(n-1)
    so the PE never idles a HAM MID window and the attention-phase exp
    (ACT-bound, ~82us) overlaps dense GEMMs.
  - V computed straight into [token, dim] via lhsT=x-tile stationary GEMM:
    kills 32 DMA_TRANSPOSE engine instrs (~40us) + 24us gpsimd copies.
  - mask matmuls N=128 (mask is all-zero beyond the diagonal 128 cols).
  - K bias dropped (softmax-invariant); V bias folded into b_proj on host
    (softmax rows sum to 1, so P@**(1 b_v)** = b_v exactly); Q bias kept.
  - reciprocal reads the denominator row straight from PSUM (no den copy).
  - out stores merged to [128,1024] on the gpsimd-hosted ring; x loads on
    sync (chunk0 fine-grained first) + gpsimd rings; scalar ring untouched
    so ACT runs exp back-to-back.

Engine split (per core):
  PE    : warmup; q/k GEMMs; V-direct GEMM; row-packed Q@K^T scores (2 heads
          via tile_position); causal -1e30 mask via ident@mask N=128 MM;
          AV with ones-column (softmax denom in PSUM row 64); proj.
  ACT   : exp(0.125*s) only (plus table preload).
  DVE   : q bias-add + k copy evacuation, V evac into vaug, reciprocal of
          denom (PSUM direct), normalize muls, proj PSUM evacuation.
  GpSimd: reciprocal broadcast; hosts x g1 loads + wqkv + out stores ring.
"""

import os
import sys

if "/opt/trn_rl_repo" not in sys.path:
    sys.path.insert(0, "/opt/trn_rl_repo")

import numpy as np

import concourse.bass as bass  # noqa: F401
import concourse.mybir as mybir
import concourse.tile as tile
from concourse import bacc
from concourse.bass_utils import run_bass_kernel_spmd
from concourse.masks import make_identity

B, S, E, H, D = 2, 2048, 1024, 16, 64
NCORES = 8
BS = B * S                   # 4096
CH = 512                     # column chunk of x^T / qkv^T / q-chunk
NCH = BS // CH               # 8 chunks
KT = S // 128                # 16 k-tiles per batch
f32 = mybir.dt.float32
bf16 = mybir.dt.bfloat16
DT = bf16
MASK_VAL = -1e30
NWARM = int(os.environ.get("NWARM", "40"))


def build_nc():
    nc = bacc.Bacc(None, target_bir_lowering=False)
    xT = nc.dram_tensor("xT", [E, BS], DT, kind="ExternalInput")
    wqkv = nc.dram_tensor("wqkv", [E, 3 * 128], DT, kind="ExternalInput")
    bqkv = nc.dram_tensor("bqkv", [128, 1], f32, kind="ExternalInput")
    wproj = nc.dram_tensor("wproj", [128, E], DT, kind="ExternalInput")
    maskb = nc.dram_tensor("maskb", [128, 128], DT, kind="ExternalInput")
    out = nc.dram_tensor("out", [BS, E], DT, kind="ExternalOutput")

    with tile.TileContext(nc) as tc:
        with (
            tc.tile_pool(name="singles", bufs=1) as singles,
            tc.tile_pool(name="ppool", bufs=6) as ppool,
            tc.tile_pool(name="npool", bufs=2) as npool,
            tc.tile_pool(name="opool", bufs=4) as opool,
            tc.tile_pool(name="ps_s", bufs=2, space="PSUM") as ps_s,
            tc.tile_pool(name="ps_y", bufs=1, space="PSUM") as ps_y,
            tc.tile_pool(name="ps_w", bufs=2, space="PSUM") as ps_w,
        ):
            # ---- persistent tiles / weight DMAs ----
            # gpsimd ring: wqkv first (needed ~5us), then batch-1 x loads.
            wqkv_sb = singles.tile([128, 8, 384], DT, tag="wqkv")
            nc.gpsimd.dma_start(
                out=wqkv_sb, in_=wqkv.rearrange("(ko ki) m -> ki ko m", ki=128)
            )
            bqkv_sb = singles.tile([128, 1], f32, tag="bqkv")
            nc.sync.dma_start(out=bqkv_sb, in_=bqkv[:, :])
            maskb_sb = singles.tile([128, 128], DT, tag="maskb")
            nc.sync.dma_start(out=maskb_sb, in_=maskb[:, :])

            wproj_sb = singles.tile([128, E], DT, tag="wproj")
            nc.gpsimd.dma_start(out=wproj_sb, in_=wproj[:, :])

            # x loads, ordered by need. chunks 0/1 fine-grained on sync so
            # qkv(0)/qkv(1) can start ~3/6us in; chunks 2-3 as [128,2,512]
            # split across both rings; chunks 4-7 as [128,4,512] on gpsimd
            # (behind the weights). Each ring is FIFO so order = priority.
            xc0, xc1 = [], []
            for k in range(8):
                t = singles.tile([128, CH], DT, tag=f"xc0_{k}")
                nc.sync.dma_start(out=t, in_=xT[k * 128:(k + 1) * 128, 0:CH])
                xc0.append(t)
            for k in range(8):
                t = singles.tile([128, CH], DT, tag=f"xc1_{k}")
                nc.sync.dma_start(out=t, in_=xT[k * 128:(k + 1) * 128, CH:2 * CH])
                xc1.append(t)
            xg23 = []
            for k in range(8):
                t = singles.tile([128, 2, CH], DT, tag=f"xg23_{k}")
                eng = nc.sync if k % 2 == 0 else nc.gpsimd
                eng.dma_start(
                    out=t,
                    in_=xT[k * 128:(k + 1) * 128, 2 * CH:4 * CH]
                    .rearrange("p (c q) -> p c q", c=2),
                )
                xg23.append(t)
            xg47 = []
            for k in range(8):
                t = singles.tile([128, 4, CH], DT, tag=f"xg47_{k}")
                nc.gpsimd.dma_start(
                    out=t,
                    in_=xT[k * 128:(k + 1) * 128, 4 * CH:8 * CH]
                    .rearrange("p (c q) -> p c q", c=4),
                )
                xg47.append(t)

            def xtile(n, k):
                if n == 0:
                    return xc0[k][:, :]
                if n == 1:
                    return xc1[k][:, :]
                if n < 4:
                    return xg23[k][:, n - 2, :]
                return xg47[k][:, n - 4, :]

            ident = singles.tile([128, 128], DT, tag="ident")
            make_identity(nc, ident[:])

            qkvT = [
                singles.tile([128, 2, CH], DT, tag=f"qkvT{n}", name=f"qkvT{n}")
                for n in range(NCH)
            ]
            # V_aug per batch: [128, kt, 130]; cols 0:64 head0 V, col 64
            # ones, 65:129 head1 V, col 129 ones. (rows = tokens of k-tile)
            vaug = [
                singles.tile([128, KT, 130], DT, tag=f"vaug{b}", name=f"vaug{b}")
                for b in range(B)
            ]
            ones_sb = singles.tile([128, KT], f32, tag="ones")
            nc.vector.memset(ones_sb[:], 1.0)
            for b in range(B):
                nc.vector.tensor_copy(out=vaug[b][:, :, 64:65], in_=ones_sb[:])
                nc.vector.tensor_copy(out=vaug[b][:, :, 129:130], in_=ones_sb[:])
            yT = [
                singles.tile([128, CH], DT, tag=f"yT{n}", name=f"yT{n}")
                for n in range(NCH)
            ]

            # exp table preload: dummy activation so the ~2.7us ACT_TABLE_LOAD
            # happens during the initial DMA prefetch, not at first real exp
            dumm = singles.tile([128, 1], f32, tag="dumm")
            nc.scalar.activation(
                out=dumm[:], in_=ones_sb[:, 0:1],
                func=mybir.ActivationFunctionType.Exp,
            )

            # HAM warmup: burn the initial x-DMA wait with throwaway matmuls
            # on a memset tile (no dependency on ident/make_identity) so the
            # PE clock is unthrottled when real work starts.
            warm = singles.tile([128, 128], DT, tag="warm")
            nc.vector.memset(warm[:], 0.0)
            warm_ps = ps_w.tile([128, 128], f32, tag="w", name="warm")
            for i in range(NWARM):
                nc.tensor.matmul(
                    warm_ps[:, 0:128], warm[:], warm[:], start=True, stop=True
                )

            def qkv_m(n, m):
                ww = ps_w.tile([128, CH], f32, tag="w", name=f"qkv{n}_{m}")
                for k in range(8):
                    nc.tensor.matmul(
                        ww[:],
                        wqkv_sb[:, k, m * 128:(m + 1) * 128],
                        xtile(n, k),
                        start=(k == 0),
                        stop=(k == 7),
                    )
                if m == 0:
                    nc.vector.tensor_scalar_add(
                        out=qkvT[n][:, 0, :], in0=ww[:],
                        scalar1=bqkv_sb[:, 0:1],
                    )
                else:
                    nc.vector.tensor_copy(out=qkvT[n][:, 1, :], in_=ww[:])

            def vdir_j(n, j):
                # V for 128-token subtile j of chunk n straight into
                # [token, vdim] layout: pv = x_tile_j^T @ Wv over 8 e-tiles.
                b, qc = n // 4, n % 4
                kt = 4 * qc + j
                pv = ps_w.tile([128, 128], f32, tag="w", name=f"v{n}_{j}")
                for k in range(8):
                    nc.tensor.matmul(
                        pv[:],
                        xtile(n, k)[:, 128 * j:128 * (j + 1)],
                        wqkv_sb[:, k, 256:384],
                        start=(k == 0),
                        stop=(k == 7),
                    )
                nc.vector.tensor_copy(out=vaug[b][:, kt, 0:64], in_=pv[:, 0:64])
                nc.vector.tensor_copy(
                    out=vaug[b][:, kt, 65:129], in_=pv[:, 64:128]
                )

            def proj_st(n, st, last=False):
                row0 = n * CH + st * 128
                o_sb = opool.tile([128, 2, CH], DT, tag="o")
                for j in range(2):
                    pp = ps_w.tile([128, CH], f32, tag="w", name=f"pj{n}_{st}_{j}")
                    nc.tensor.matmul(
                        pp[:],
                        yT[n][:, st * 128:(st + 1) * 128],
                        wproj_sb[:, j * CH:(j + 1) * CH],
                        start=True,
                        stop=True,
                    )
                    if last and j == 1:
                        # ACT is idle at the tail — split the evacuation
                        nc.scalar.copy(out=o_sb[:, j, :], in_=pp[:])
                    else:
                        nc.vector.tensor_copy(out=o_sb[:, j, :], in_=pp[:])
                eng = nc.gpsimd if st % 2 == 0 else nc.sync
                eng.dma_start(out=out[row0:row0 + 128, :], in_=o_sb[:, :, :])

            def attention_qc(b, qc, dense_units):
                nq = b * 4 + qc
                ktmax = 4 * (qc + 1)
                py = [
                    ps_y.tile([65, CH], f32, tag=f"y{h}", name=f"py{b}_{qc}_{h}")
                    for h in range(2)
                ]

                def emit_av(kt, pt_sb, off):
                    for h in range(2):
                        nc.tensor.matmul(
                            py[h][:, off:CH],
                            vaug[b][:, kt, h * 65:h * 65 + 65],
                            pt_sb[:, h, off:CH],
                            start=(kt == 0),
                            stop=(kt == ktmax - 1),
                            skip_group_check=True,
                        )

                pending = []  # software pipeline: AV(t) emitted after exp(t+2)
                emitted = 0
                for kt in range(ktmax):
                    o = kt - 4 * qc if kt >= 4 * qc else None  # diagonal index
                    off = 128 * o if o is not None else 0
                    nk = b * 4 + kt // 4
                    offk = (kt % 4) * 128
                    pg = ps_s.tile([128, 2, CH], f32, tag="s")
                    for h in range(2):
                        hb = h * 64
                        nc.tensor.matmul(
                            pg[:, h, off:CH],
                            qkvT[nk][hb:hb + 64, 1, offk:offk + 128],
                            qkvT[nq][hb:hb + 64, 0, off:CH],
                            start=True,
                            stop=(o is None),
                            tile_position=(hb, 0),
                            skip_group_check=True,
                        )
                    if o is not None:
                        # add -1e30 causal mask into the diagonal 128 cols of
                        # the score PSUM: += I^T @ M (mask is 0 past them)
                        for h in range(2):
                            nc.tensor.matmul(
                                pg[:, h, off:off + 128],
                                ident[:],
                                maskb_sb[:, :],
                                start=False,
                                stop=True,
                                skip_group_check=True,
                            )
                    pt_sb = ppool.tile([128, 2, CH], DT, tag="pT")
                    nc.scalar.activation(
                        out=pt_sb[:, :, off:CH],
                        in_=pg[:, :, off:CH],
                        func=mybir.ActivationFunctionType.Exp,
                        scale=0.125,
                    )
                    # pace independent dense work (proj of chunk n-1, qkv/V of
                    # chunk n+1) through the exp-bound loop so the PE never
                    # starves waiting for the ACT engine
                    target = (kt + 1) * len(dense_units) // ktmax
                    while emitted < target:
                        dense_units[emitted]()
                        emitted += 1
                    pending.append((kt, pt_sb, off))
                    if len(pending) > 2:
                        emit_av(*pending.pop(0))
                while emitted < len(dense_units):
                    dense_units[emitted]()
                    emitted += 1
                for p in pending:
                    emit_av(*p)
                # normalize both heads: copy den to partition 0 (regular DVE
                # copy handles the partition remap; the custom-DVE recip does
                # not), rec = 1/den; broadcast on gpsimd; yT = py * rec
                den2 = npool.tile([1, 2, CH], f32, tag="den")
                for h in range(2):
                    nc.vector.tensor_copy(out=den2[:, h, :], in_=py[h][64:65, :])
                rec2 = npool.tile([1, 2, CH], f32, tag="rec")
                nc.vector.reciprocal_approx_fast(out=rec2[:], in_=den2[:])
                bc = npool.tile([64, 2, CH], f32, tag="bc")
                nc.gpsimd.partition_broadcast(out_ap=bc[:], in_ap=rec2[:])
                for h in range(2):
                    nc.vector.tensor_mul(
                        out=yT[nq][h * 64:h * 64 + 64, :],
                        in0=py[h][0:64, :],
                        in1=bc[:, h, :],
                    )

            # engine streams execute IN ORDER; emission order is the PE
            # program. Chunk n's attention (exp-paced on ACT) is interleaved
            # at sub-chunk granularity with proj(n-1) and qkv/V of chunk n+1
            # so the PE stays busy (and HAM-warm) through the exp waits.
            qkv_m(0, 0)
            qkv_m(0, 1)
            for j in range(4):
                vdir_j(0, j)
            for n in range(NCH):
                b, qc = n // 4, n % 4
                units = []
                if n + 1 < NCH:
                    units += [lambda m=m, nn=n + 1: qkv_m(nn, m) for m in range(2)]
                    units += [lambda j=j, nn=n + 1: vdir_j(nn, j) for j in range(4)]
                if n > 0:
                    units += [lambda st=st, nn=n - 1: proj_st(nn, st) for st in range(4)]
                attention_qc(b, qc, units)
            for st in range(4):
                proj_st(NCH - 1, st, last=True)

    nc.finalize()
    return nc


def make_core_inputs(x, W_attn, b_attn, W_proj):
    """Host-side sharding: slice per-core weights, transpose x, build mask."""
    np_dt = mybir.dt.np(DT)
    xT = np.ascontiguousarray(x.reshape(BS, E).T).astype(np_dt)  # [E, BS]

    # single additive causal mask for a diagonal 128x128 block:
    # valid iff j >= i (j = within-block q, i = k-in-tile)
    i = np.arange(128)[:, None]
    j = np.arange(128)[None, :]
    maskb = np.ascontiguousarray(
        np.where(j >= i, 0.0, MASK_VAL).astype(np_dt)
    )  # [128, 128]

    in_maps = []
    for c in range(NCORES):
        cols = slice(128 * c, 128 * (c + 1))
        wqkv = np.ascontiguousarray(
            np.concatenate(
                [W_attn[:, cols], W_attn[:, E:][:, cols], W_attn[:, 2 * E:][:, cols]],
                axis=1,
            )
        ).astype(np_dt)  # [E, 384]
        bq = np.ascontiguousarray(
            b_attn[cols].astype(np.float32)[:, None]
        )  # [128, 1] — q bias only (k bias is softmax-invariant, v bias is
        # folded into b_proj on the host)
        wp = np.ascontiguousarray(W_proj[128 * c:128 * (c + 1), :]).astype(np_dt)
        in_maps.append(
            {"xT": xT, "wqkv": wqkv, "bqkv": bq, "wproj": wp, "maskb": maskb}
        )
    return in_maps


_NC_CACHE = None


def kernel_run(inputs, trace=False):
    """Run the bass kernel; returns (full_output, BassKernelResults)."""
    global _NC_CACHE
    x = np.asarray(inputs["x"], dtype=np.float32)
    W_attn = np.asarray(inputs["W_attn"], dtype=np.float32)
    b_attn = np.asarray(inputs["b_attn"], dtype=np.float32)
    W_proj = np.asarray(inputs["W_proj"], dtype=np.float32)
    b_proj = np.asarray(inputs["b_proj"], dtype=np.float32)

    if _NC_CACHE is None:
        _NC_CACHE = build_nc()
    nc = _NC_CACHE

    in_maps = make_core_inputs(x, W_attn, b_attn, W_proj)
    res = run_bass_kernel_spmd(
        nc, in_maps, core_ids=list(range(NCORES)), trace=trace
    )
    acc = np.zeros((BS, E), dtype=np.float64)
    for r in res.results:
        acc += np.asarray(r["out"], dtype=np.float64)
    # b_proj_eff: v-bias contribution folds exactly through softmax + proj
    b_eff = b_proj.astype(np.float64) + b_attn[2 * E:].astype(
        np.float64
    ) @ W_proj.astype(np.float64)
    y = (acc + b_eff).astype(np.float32).reshape(B, S, E)
    return y, res


def kernel(**inputs):
    y, _ = kernel_run(inputs, trace=False)
    return y


if __name__ == "__main__":
    rng = np.random.default_rng(0)
    scale = 1.0 / np.sqrt(E)
    inputs = {
        "x": rng.standard_normal((B, S, E), dtype=np.float32),
        "W_attn": rng.standard_normal((E, 3 * E), dtype=np.float32) * scale,
        "b_attn": rng.standard_normal((3 * E,), dtype=np.float32) * 0.02,
        "W_proj": rng.standard_normal((E, E), dtype=np.float32) * scale,
        "b_proj": rng.standard_normal((E,), dtype=np.float32) * 0.02,
    }
    y = kernel(**inputs)
    print("kernel output", y.shape, y.dtype, float(np.abs(y).mean()))


# revision 11
# speedup vs baseline: 1.0879x; 1.0879x over previous
"""Causal self-attention (B=2, S=2048, E=1024, H=16, D=64) on 8 trn2 NeuronCores.

Sharding: tensor-parallel over heads — 2 heads per core. Each core computes
q^T,k^T for its 2 heads, V directly in [token, dim] layout via a separate
GEMM (no XBAR transposes), runs causal attention, and multiplies by its
128-row slice of W_proj, producing a partial [4096, 1024] output (bf16).
The host sums the 8 partials and adds b_proj_eff.

v2 changes vs v1 (214999 ns):
  - per-chunk interleave: qkv(n) -> vdir(n) -> attention(b,qc) -> proj(n-1)
    so the PE never idles a HAM MID window and the attention-phase exp
    (ACT-bound, ~82us) overlaps dense GEMMs.
  - V computed straight into [token, dim] via lhsT=x-tile stationary GEMM:
    kills 32 DMA_TRANSPOSE engine instrs (~40us) + 24us gpsimd copies.
  - mask matmuls N=128 (mask is all-zero beyond the diagonal 128 cols).
  - K bias dropped (softmax-invariant); V bias folded into b_proj on host
    (softmax rows sum to 1, so P@**(1 b_v)** = b_v exactly); Q bias kept.
  - reciprocal reads the denominator row straight from PSUM (no den copy).
  - out stores merged to [128,1024] on the gpsimd-hosted ring; x loads on
    sync (chunk0 fine-grained first) + gpsimd rings; scalar ring untouched
    so ACT runs exp back-to-back.

Engine split (per core):
  PE    : warmup; q/k GEMMs; V-direct GEMM; row-packed Q@K^T scores (2 heads
          via tile_position); causal -1e30 mask via ident@mask N=128 MM;
          AV with ones-column (softmax denom in PSUM row 64); proj.
  ACT   : exp(0.125*s) only (plus table preload).
  DVE   : q bias-add + k copy evacuation, V evac into vaug, reciprocal of
          denom (PSUM direct), normalize muls, proj PSUM evacuation.
  GpSimd: reciprocal broadcast; hosts x g1 loads + wqkv + out stores ring.
"""

import os
import sys

if "/opt/trn_rl_repo" not in sys.path:
    sys.path.insert(0, "/opt/trn_rl_repo")

import numpy as np

import concourse.bass as bass  # noqa: F401
import concourse.mybir as mybir
import concourse.tile as tile
from concourse import bacc
from concourse.bass_utils import run_bass_kernel_spmd
from concourse.masks import make_identity

B, S, E, H, D = 2, 2048, 1024, 16, 64
NCORES = 8
BS = B * S                   # 4096
CH = 512                     # column chunk of x^T / qkv^T / q-chunk
NCH = BS // CH               # 8 chunks
KT = S // 128                # 16 k-tiles per batch
f32 = mybir.dt.float32
bf16 = mybir.dt.bfloat16
DT = bf16
MASK_VAL = -1e30
NWARM = int(os.environ.get("NWARM", "40"))


def build_nc():
    nc = bacc.Bacc(None, target_bir_lowering=False)
    xT = nc.dram_tensor("xT", [E, BS], DT, kind="ExternalInput")
    wqkv = nc.dram_tensor("wqkv", [E, 3 * 128], DT, kind="ExternalInput")
    bqkv = nc.dram_tensor("bqkv", [128, 1], f32, kind="ExternalInput")
    wproj = nc.dram_tensor("wproj", [128, E], DT, kind="ExternalInput")
    maskb = nc.dram_tensor("maskb", [128, 128], DT, kind="ExternalInput")
    out = nc.dram_tensor("out", [BS, E], DT, kind="ExternalOutput")

    with tile.TileContext(nc) as tc:
        with (
            tc.tile_pool(name="singles", bufs=1) as singles,
            tc.tile_pool(name="ppool", bufs=6) as ppool,
            tc.tile_pool(name="npool", bufs=2) as npool,
            tc.tile_pool(name="opool", bufs=4) as opool,
            tc.tile_pool(name="ps_s", bufs=2, space="PSUM") as ps_s,
            tc.tile_pool(name="ps_y", bufs=1, space="PSUM") as ps_y,
            tc.tile_pool(name="ps_w", bufs=2, space="PSUM") as ps_w,
        ):
            # ---- persistent tiles / weight DMAs ----
            # gpsimd ring: wqkv first (needed ~5us), then batch-1 x loads.
            wqkv_sb = singles.tile([128, 8, 384], DT, tag="wqkv")
            nc.gpsimd.dma_start(
                out=wqkv_sb, in_=wqkv.rearrange("(ko ki) m -> ki ko m", ki=128)
            )
            bqkv_sb = singles.tile([128, 1], f32, tag="bqkv")
            nc.sync.dma_start(out=bqkv_sb, in_=bqkv[:, :])
            maskb_sb = singles.tile([128, 128], DT, tag="maskb")
            nc.sync.dma_start(out=maskb_sb, in_=maskb[:, :])

            wproj_sb = singles.tile([128, E], DT, tag="wproj")
            nc.gpsimd.dma_start(out=wproj_sb, in_=wproj[:, :])

            # x loads: the whole phase is per-core-HBM-bandwidth-bound
            # (~25us for 9MB), so order them exactly in consumption order on
            # one ring, one DMA per chunk (3-dim AP gathers all 8 k-blocks).
            xch = []
            for n in range(NCH):
                t = singles.tile([128, 8, CH], DT, tag=f"xch{n}")
                nc.sync.dma_start(
                    out=t,
                    in_=xT[:, n * CH:(n + 1) * CH]
                    .rearrange("(ko ki) q -> ki ko q", ki=128),
                )
                xch.append(t)

            def xtile(n, k):
                return xch[n][:, k, :]

            ident = singles.tile([128, 128], DT, tag="ident")
            make_identity(nc, ident[:])

            qkvT = [
                singles.tile([128, 2, CH], DT, tag=f"qkvT{n}", name=f"qkvT{n}")
                for n in range(NCH)
            ]
            # V_aug per batch: [128, kt, 130]; cols 0:64 head0 V, col 64
            # ones, 65:129 head1 V, col 129 ones. (rows = tokens of k-tile)
            vaug = [
                singles.tile([128, KT, 130], DT, tag=f"vaug{b}", name=f"vaug{b}")
                for b in range(B)
            ]
            ones_sb = singles.tile([128, KT], f32, tag="ones")
            nc.vector.memset(ones_sb[:], 1.0)
            for b in range(B):
                nc.vector.tensor_copy(out=vaug[b][:, :, 64:65], in_=ones_sb[:])
                nc.vector.tensor_copy(out=vaug[b][:, :, 129:130], in_=ones_sb[:])
            yT = [
                singles.tile([128, CH], DT, tag=f"yT{n}", name=f"yT{n}")
                for n in range(NCH)
            ]

            # exp table preload: dummy activation so the ~2.7us ACT_TABLE_LOAD
            # happens during the initial DMA prefetch, not at first real exp
            dumm = singles.tile([128, 1], f32, tag="dumm")
            nc.scalar.activation(
                out=dumm[:], in_=ones_sb[:, 0:1],
                func=mybir.ActivationFunctionType.Exp,
            )

            # HAM warmup: burn the initial x-DMA wait with throwaway matmuls
            # on a memset tile (no dependency on ident/make_identity) so the
            # PE clock is unthrottled when real work starts.
            warm = singles.tile([128, 128], DT, tag="warm")
            nc.vector.memset(warm[:], 0.0)
            warm_ps = ps_w.tile([128, 128], f32, tag="w", name="warm")
            for i in range(NWARM):
                nc.tensor.matmul(
                    warm_ps[:, 0:128], warm[:], warm[:], start=True, stop=True
                )

            def qkv_m(n, m):
                ww = ps_w.tile([128, CH], f32, tag="w", name=f"qkv{n}_{m}")
                for k in range(8):
                    nc.tensor.matmul(
                        ww[:],
                        wqkv_sb[:, k, m * 128:(m + 1) * 128],
                        xtile(n, k),
                        start=(k == 0),
                        stop=(k == 7),
                    )
                if m == 0:
                    nc.vector.tensor_scalar_add(
                        out=qkvT[n][:, 0, :], in0=ww[:],
                        scalar1=bqkv_sb[:, 0:1],
                    )
                else:
                    nc.vector.tensor_copy(out=qkvT[n][:, 1, :], in_=ww[:])

            def vdir_j(n, j):
                # V for 128-token subtile j of chunk n straight into
                # [token, vdim] layout: pv = x_tile_j^T @ Wv over 8 e-tiles.
                b, qc = n // 4, n % 4
                kt = 4 * qc + j
                pv = ps_w.tile([128, 128], f32, tag="w", name=f"v{n}_{j}")
                for k in range(8):
                    nc.tensor.matmul(
                        pv[:],
                        xtile(n, k)[:, 128 * j:128 * (j + 1)],
                        wqkv_sb[:, k, 256:384],
                        start=(k == 0),
                        stop=(k == 7),
                    )
                nc.vector.tensor_copy(out=vaug[b][:, kt, 0:64], in_=pv[:, 0:64])
                nc.vector.tensor_copy(
                    out=vaug[b][:, kt, 65:129], in_=pv[:, 64:128]
                )

            def proj_st(n, st, last=False):
                row0 = n * CH + st * 128
                o_sb = opool.tile([128, 2, CH], DT, tag="o")
                for j in range(2):
                    pp = ps_w.tile([128, CH], f32, tag="w", name=f"pj{n}_{st}_{j}")
                    nc.tensor.matmul(
                        pp[:],
                        yT[n][:, st * 128:(st + 1) * 128],
                        wproj_sb[:, j * CH:(j + 1) * CH],
                        start=True,
                        stop=True,
                    )
                    if last and j == 1:
                        # ACT is idle at the tail — split the evacuation
                        nc.scalar.copy(out=o_sb[:, j, :], in_=pp[:])
                    else:
                        nc.vector.tensor_copy(out=o_sb[:, j, :], in_=pp[:])
                eng = nc.gpsimd if st % 2 == 0 else nc.sync
                eng.dma_start(out=out[row0:row0 + 128, :], in_=o_sb[:, :, :])

            def attention_qc(b, qc, dense_units):
                nq = b * 4 + qc
                ktmax = 4 * (qc + 1)
                py = [
                    ps_y.tile([65, CH], f32, tag=f"y{h}", name=f"py{b}_{qc}_{h}")
                    for h in range(2)
                ]

                def emit_av(kt, pt_sb, off):
                    for h in range(2):
                        nc.tensor.matmul(
                            py[h][:, off:CH],
                            vaug[b][:, kt, h * 65:h * 65 + 65],
                            pt_sb[:, h, off:CH],
                            start=(kt == 0),
                            stop=(kt == ktmax - 1),
                            skip_group_check=True,
                        )

                pending = []  # software pipeline: AV(t) emitted after exp(t+2)
                emitted = 0
                for kt in range(ktmax):
                    o = kt - 4 * qc if kt >= 4 * qc else None  # diagonal index
                    off = 128 * o if o is not None else 0
                    nk = b * 4 + kt // 4
                    offk = (kt % 4) * 128
                    pg = ps_s.tile([128, 2, CH], f32, tag="s")
                    for h in range(2):
                        hb = h * 64
                        nc.tensor.matmul(
                            pg[:, h, off:CH],
                            qkvT[nk][hb:hb + 64, 1, offk:offk + 128],
                            qkvT[nq][hb:hb + 64, 0, off:CH],
                            start=True,
                            stop=(o is None),
                            tile_position=(hb, 0),
                            skip_group_check=True,
                        )
                    if o is not None:
                        # add -1e30 causal mask into the diagonal 128 cols of
                        # the score PSUM: += I^T @ M (mask is 0 past them)
                        for h in range(2):
                            nc.tensor.matmul(
                                pg[:, h, off:off + 128],
                                ident[:],
                                maskb_sb[:, :],
                                start=False,
                                stop=True,
                                skip_group_check=True,
                            )
                    pt_sb = ppool.tile([128, 2, CH], DT, tag="pT")
                    nc.scalar.activation(
                        out=pt_sb[:, :, off:CH],
                        in_=pg[:, :, off:CH],
                        func=mybir.ActivationFunctionType.Exp,
                        scale=0.125,
                    )
                    # pace independent dense work (proj of chunk n-1, qkv/V of
                    # chunk n+1) through the exp-bound loop so the PE never
                    # starves waiting for the ACT engine
                    target = (kt + 1) * len(dense_units) // ktmax
                    while emitted < target:
                        dense_units[emitted]()
                        emitted += 1
                    pending.append((kt, pt_sb, off))
                    if len(pending) > 2:
                        emit_av(*pending.pop(0))
                while emitted < len(dense_units):
                    dense_units[emitted]()
                    emitted += 1
                for p in pending:
                    emit_av(*p)
                # normalize both heads: copy den to partition 0 (regular DVE
                # copy handles the partition remap; the custom-DVE recip does
                # not), rec = 1/den; broadcast on gpsimd; yT = py * rec
                den2 = npool.tile([1, 2, CH], f32, tag="den")
                for h in range(2):
                    nc.vector.tensor_copy(out=den2[:, h, :], in_=py[h][64:65, :])
                rec2 = npool.tile([1, 2, CH], f32, tag="rec")
                nc.vector.reciprocal_approx_fast(out=rec2[:], in_=den2[:])
                bc = npool.tile([64, 2, CH], f32, tag="bc")
                nc.gpsimd.partition_broadcast(out_ap=bc[:], in_ap=rec2[:])
                for h in range(2):
                    nc.vector.tensor_mul(
                        out=yT[nq][h * 64:h * 64 + 64, :],
                        in0=py[h][0:64, :],
                        in1=bc[:, h, :],
                    )

            # engine streams execute IN ORDER; emission order is the PE
            # program. Chunk n's attention (exp-paced on ACT) is interleaved
            # at sub-chunk granularity with proj(n-1) and qkv/V of chunk n+1
            # so the PE stays busy (and HAM-warm) through the exp waits.
            # proj of chunk p is deferred into the chunk listed here: the
            # qc=3 chunks are exp(ACT)-paced with PE slack (chunk 7 has no
            # qkv fill at all), while early chunks are PE/DMA-bound — parking
            # proj work late fills the exp waits instead of adding wall.
            proj_in = {3: [0], 4: [1], 5: [2], 6: [3], 7: [4, 5, 6]}
            qkv_m(0, 0)
            qkv_m(0, 1)
            for j in range(4):
                vdir_j(0, j)
            for n in range(NCH):
                b, qc = n // 4, n % 4
                units = []
                if n + 1 < NCH:
                    units += [lambda m=m, nn=n + 1: qkv_m(nn, m) for m in range(2)]
                    units += [lambda j=j, nn=n + 1: vdir_j(nn, j) for j in range(4)]
                for p in proj_in.get(n, []):
                    units += [lambda st=st, pp=p: proj_st(pp, st) for st in range(4)]
                attention_qc(b, qc, units)
            for st in range(4):
                proj_st(NCH - 1, st, last=True)

    nc.finalize()
    return nc


def make_core_inputs(x, W_attn, b_attn, W_proj):
    """Host-side sharding: slice per-core weights, transpose x, build mask."""
    np_dt = mybir.dt.np(DT)
    xT = np.ascontiguousarray(x.reshape(BS, E).T).astype(np_dt)  # [E, BS]

    # single additive causal mask for a diagonal 128x128 block:
    # valid iff j >= i (j = within-block q, i = k-in-tile)
    i = np.arange(128)[:, None]
    j = np.arange(128)[None, :]
    maskb = np.ascontiguousarray(
        np.where(j >= i, 0.0, MASK_VAL).astype(np_dt)
    )  # [128, 128]

    in_maps = []
    for c in range(NCORES):
        cols = slice(128 * c, 128 * (c + 1))
        wqkv = np.ascontiguousarray(
            np.concatenate(
                [W_attn[:, cols], W_attn[:, E:][:, cols], W_attn[:, 2 * E:][:, cols]],
                axis=1,
            )
        ).astype(np_dt)  # [E, 384]
        bq = np.ascontiguousarray(
            b_attn[cols].astype(np.float32)[:, None]
        )  # [128, 1] — q bias only (k bias is softmax-invariant, v bias is
        # folded into b_proj on the host)
        wp = np.ascontiguousarray(W_proj[128 * c:128 * (c + 1), :]).astype(np_dt)
        in_maps.append(
            {"xT": xT, "wqkv": wqkv, "bqkv": bq, "wproj": wp, "maskb": maskb}
        )
    return in_maps


_NC_CACHE = None


def kernel_run(inputs, trace=False):
    """Run the bass kernel; returns (full_output, BassKernelResults)."""
    global _NC_CACHE
    x = np.asarray(inputs["x"], dtype=np.float32)
    W_attn = np.asarray(inputs["W_attn"], dtype=np.float32)
    b_attn = np.asarray(inputs["b_attn"], dtype=np.float32)
    W_proj = np.asarray(inputs["W_proj"], dtype=np.float32)
    b_proj = np.asarray(inputs["b_proj"], dtype=np.float32)

    if _NC_CACHE is None:
        _NC_CACHE = build_nc()
    nc = _NC_CACHE

    in_maps = make_core_inputs(x, W_attn, b_attn, W_proj)
    res = run_bass_kernel_spmd(
        nc, in_maps, core_ids=list(range(NCORES)), trace=trace
    )
    acc = np.zeros((BS, E), dtype=np.float64)
    for r in res.results:
        acc += np.asarray(r["out"], dtype=np.float64)
    # b_proj_eff: v-bias contribution folds exactly through softmax + proj
    b_eff = b_proj.astype(np.float64) + b_attn[2 * E:].astype(
        np.float64
    ) @ W_proj.astype(np.float64)
    y = (acc + b_eff).astype(np.float32).reshape(B, S, E)
    return y, res


def kernel(**inputs):
    y, _ = kernel_run(inputs, trace=False)
    return y


if __name__ == "__main__":
    rng = np.random.default_rng(0)
    scale = 1.0 / np.sqrt(E)
    inputs = {
        "x": rng.standard_normal((B, S, E), dtype=np.float32),
        "W_attn": rng.standard_normal((E, 3 * E), dtype=np.float32) * scale,
        "b_attn": rng.standard_normal((3 * E,), dtype=np.float32) * 0.02,
        "W_proj": rng.standard_normal((E, E), dtype=np.float32) * scale,
        "b_proj": rng.standard_normal((E,), dtype=np.float32) * 0.02,
    }
    y = kernel(**inputs)
    print("kernel output", y.shape, y.dtype, float(np.abs(y).mean()))


# revision 14
# speedup vs baseline: 1.1026x; 1.0135x over previous
"""Causal self-attention (B=2, S=2048, E=1024, H=16, D=64) on 8 trn2 NeuronCores.

Sharding: tensor-parallel over heads — 2 heads per core. Each core computes
q^T,k^T for its 2 heads, V directly in [token, dim] layout via a separate
GEMM (no XBAR transposes), runs causal attention, and multiplies by its
128-row slice of W_proj, producing a partial [4096, 1024] output (bf16).
The host sums the 8 partials and adds b_proj_eff.

v2 changes vs v1 (214999 ns):
  - per-chunk interleave: qkv(n) -> vdir(n) -> attention(b,qc) -> proj(n-1)
    so the PE never idles a HAM MID window and the attention-phase exp
    (ACT-bound, ~82us) overlaps dense GEMMs.
  - V computed straight into [token, dim] via lhsT=x-tile stationary GEMM:
    kills 32 DMA_TRANSPOSE engine instrs (~40us) + 24us gpsimd copies.
  - mask matmuls N=128 (mask is all-zero beyond the diagonal 128 cols).
  - K bias dropped (softmax-invariant); V bias folded into b_proj on host
    (softmax rows sum to 1, so P@**(1 b_v)** = b_v exactly); Q bias kept.
  - reciprocal reads the denominator row straight from PSUM (no den copy).
  - out stores merged to [128,1024] on the gpsimd-hosted ring; x loads on
    sync (chunk0 fine-grained first) + gpsimd rings; scalar ring untouched
    so ACT runs exp back-to-back.

Engine split (per core):
  PE    : warmup; q/k GEMMs; V-direct GEMM; row-packed Q@K^T scores (2 heads
          via tile_position); causal -1e30 mask via ident@mask N=128 MM;
          AV with ones-column (softmax denom in PSUM row 64); proj.
  ACT   : exp(0.125*s) only (plus table preload).
  DVE   : q bias-add + k copy evacuation, V evac into vaug, reciprocal of
          denom (PSUM direct), normalize muls, proj PSUM evacuation.
  GpSimd: reciprocal broadcast; hosts x g1 loads + wqkv + out stores ring.
"""

import os
import sys

if "/opt/trn_rl_repo" not in sys.path:
    sys.path.insert(0, "/opt/trn_rl_repo")

import numpy as np

import concourse.bass as bass  # noqa: F401
import concourse.mybir as mybir
import concourse.tile as tile
from concourse import bacc
from concourse.bass_utils import run_bass_kernel_spmd
from concourse.masks import make_identity

B, S, E, H, D = 2, 2048, 1024, 16, 64
NCORES = 8
BS = B * S                   # 4096
CH = 512                     # column chunk of x^T / qkv^T / q-chunk
NCH = BS // CH               # 8 chunks
KT = S // 128                # 16 k-tiles per batch
f32 = mybir.dt.float32
bf16 = mybir.dt.bfloat16
DT = bf16
MASK_VAL = -1e30
NWARM = int(os.environ.get("NWARM", "88"))


def build_nc():
    nc = bacc.Bacc(None, target_bir_lowering=False)
    xT = nc.dram_tensor("xT", [E, BS], DT, kind="ExternalInput")
    wqkv = nc.dram_tensor("wqkv", [E, 3 * 128], DT, kind="ExternalInput")
    bqkv = nc.dram_tensor("bqkv", [128, 1], f32, kind="ExternalInput")
    wproj = nc.dram_tensor("wproj", [128, E], DT, kind="ExternalInput")
    maskb = nc.dram_tensor("maskb", [128, 128], DT, kind="ExternalInput")
    out = nc.dram_tensor("out", [BS, E], DT, kind="ExternalOutput")

    with tile.TileContext(nc) as tc:
        with (
            tc.tile_pool(name="singles", bufs=1) as singles,
            tc.tile_pool(name="ppool", bufs=6) as ppool,
            tc.tile_pool(name="npool", bufs=2) as npool,
            tc.tile_pool(name="opool", bufs=4) as opool,
            tc.tile_pool(name="ps_s", bufs=2, space="PSUM") as ps_s,
            tc.tile_pool(name="ps_y", bufs=1, space="PSUM") as ps_y,
            tc.tile_pool(name="ps_w", bufs=2, space="PSUM") as ps_w,
        ):
            # ---- persistent tiles / weight DMAs ----
            # gpsimd ring: wqkv first (needed ~5us), then batch-1 x loads.
            wqkv_sb = singles.tile([128, 8, 384], DT, tag="wqkv")
            nc.gpsimd.dma_start(
                out=wqkv_sb, in_=wqkv.rearrange("(ko ki) m -> ki ko m", ki=128)
            )
            bqkv_sb = singles.tile([128, 1], f32, tag="bqkv")
            nc.sync.dma_start(out=bqkv_sb, in_=bqkv[:, :])
            maskb_sb = singles.tile([128, 128], DT, tag="maskb")
            nc.sync.dma_start(out=maskb_sb, in_=maskb[:, :])

            wproj_sb = singles.tile([128, E], DT, tag="wproj")
            nc.gpsimd.dma_start(out=wproj_sb, in_=wproj[:, :])

            # x loads: the whole phase is per-core-HBM-bandwidth-bound
            # (~25us for 9MB), so order them exactly in consumption order on
            # one ring, one DMA per chunk (3-dim AP gathers all 8 k-blocks).
            xch = []
            for n in range(NCH):
                t = singles.tile([128, 8, CH], DT, tag=f"xch{n}")
                nc.sync.dma_start(
                    out=t,
                    in_=xT[:, n * CH:(n + 1) * CH]
                    .rearrange("(ko ki) q -> ki ko q", ki=128),
                )
                xch.append(t)

            def xtile(n, k):
                return xch[n][:, k, :]

            ident = singles.tile([128, 128], DT, tag="ident")
            make_identity(nc, ident[:])

            qkvT = [
                singles.tile([128, 2, CH], DT, tag=f"qkvT{n}", name=f"qkvT{n}")
                for n in range(NCH)
            ]
            # V_aug per batch: [128, kt, 130]; cols 0:64 head0 V, col 64
            # ones, 65:129 head1 V, col 129 ones. (rows = tokens of k-tile)
            vaug = [
                singles.tile([128, KT, 130], DT, tag=f"vaug{b}", name=f"vaug{b}")
                for b in range(B)
            ]
            ones_sb = singles.tile([128, KT], f32, tag="ones")
            nc.vector.memset(ones_sb[:], 1.0)
            for b in range(B):
                nc.vector.tensor_copy(out=vaug[b][:, :, 64:65], in_=ones_sb[:])
                nc.vector.tensor_copy(out=vaug[b][:, :, 129:130], in_=ones_sb[:])
            yT = [
                singles.tile([128, CH], DT, tag=f"yT{n}", name=f"yT{n}")
                for n in range(NCH)
            ]

            # exp table preload: dummy activation so the ~2.7us ACT_TABLE_LOAD
            # happens during the initial DMA prefetch, not at first real exp
            dumm = singles.tile([128, 1], f32, tag="dumm")
            nc.scalar.activation(
                out=dumm[:], in_=ones_sb[:, 0:1],
                func=mybir.ActivationFunctionType.Exp,
            )

            # HAM warmup: burn the initial x-DMA wait with throwaway matmuls
            # on a memset tile (no dependency on ident/make_identity) so the
            # PE clock is unthrottled when real work starts.
            warm = singles.tile([128, 128], DT, tag="warm")
            nc.vector.memset(warm[:], 0.0)
            warm_ps = ps_w.tile([128, 128], f32, tag="w", name="warm")
            for i in range(NWARM):
                nc.tensor.matmul(
                    warm_ps[:, 0:128], warm[:], warm[:], start=True, stop=True
                )

            def qkv_m(n, m):
                ww = ps_w.tile([128, CH], f32, tag="w", name=f"qkv{n}_{m}")
                for k in range(8):
                    nc.tensor.matmul(
                        ww[:],
                        wqkv_sb[:, k, m * 128:(m + 1) * 128],
                        xtile(n, k),
                        start=(k == 0),
                        stop=(k == 7),
                    )
                if m == 0:
                    nc.vector.tensor_scalar_add(
                        out=qkvT[n][:, 0, :], in0=ww[:],
                        scalar1=bqkv_sb[:, 0:1],
                    )
                else:
                    nc.vector.tensor_copy(out=qkvT[n][:, 1, :], in_=ww[:])

            def vdir_j(n, j):
                # V for 128-token subtile j of chunk n straight into
                # [token, vdim] layout: pv = x_tile_j^T @ Wv over 8 e-tiles.
                b, qc = n // 4, n % 4
                kt = 4 * qc + j
                pv = ps_w.tile([128, 128], f32, tag="w", name=f"v{n}_{j}")
                for k in range(8):
                    nc.tensor.matmul(
                        pv[:],
                        xtile(n, k)[:, 128 * j:128 * (j + 1)],
                        wqkv_sb[:, k, 256:384],
                        start=(k == 0),
                        stop=(k == 7),
                    )
                nc.vector.tensor_copy(out=vaug[b][:, kt, 0:64], in_=pv[:, 0:64])
                nc.vector.tensor_copy(
                    out=vaug[b][:, kt, 65:129], in_=pv[:, 64:128]
                )

            def proj_st(n, st, last=False):
                row0 = n * CH + st * 128
                o_sb = opool.tile([128, 2, CH], DT, tag="o")
                for j in range(2):
                    if last:
                        # tail: scores are done — borrow the idle ps_s banks
                        # so the 8 final MMs aren't strangled by the 2-buffer
                        # ps_w round-trip, and split evac across DVE + ACT
                        ps = ps_s.tile([128, 2, CH], f32, tag="s")
                        pp = ps[:, 0, :]
                    else:
                        pp = ps_w.tile(
                            [128, CH], f32, tag="w", name=f"pj{n}_{st}_{j}"
                        )[:]
                    nc.tensor.matmul(
                        pp,
                        yT[n][:, st * 128:(st + 1) * 128],
                        wproj_sb[:, j * CH:(j + 1) * CH],
                        start=True,
                        stop=True,
                    )
                    if last and j == 1:
                        nc.scalar.copy(out=o_sb[:, j, :], in_=pp)
                    else:
                        nc.vector.tensor_copy(out=o_sb[:, j, :], in_=pp)
                eng = nc.gpsimd if st % 2 == 0 else nc.sync
                eng.dma_start(out=out[row0:row0 + 128, :], in_=o_sb[:, :, :])

            def attention_qc(b, qc, dense_units):
                nq = b * 4 + qc
                ktmax = 4 * (qc + 1)
                py = [
                    ps_y.tile([65, CH], f32, tag=f"y{h}", name=f"py{b}_{qc}_{h}")
                    for h in range(2)
                ]

                def emit_av(kt, pt_sb, off):
                    for h in range(2):
                        nc.tensor.matmul(
                            py[h][:, off:CH],
                            vaug[b][:, kt, h * 65:h * 65 + 65],
                            pt_sb[:, h, off:CH],
                            start=(kt == 0),
                            stop=(kt == ktmax - 1),
                            skip_group_check=True,
                        )

                pending = []  # software pipeline: AV(t) emitted after exp(t+2)
                emitted = 0
                for kt in range(ktmax):
                    o = kt - 4 * qc if kt >= 4 * qc else None  # diagonal index
                    off = 128 * o if o is not None else 0
                    nk = b * 4 + kt // 4
                    offk = (kt % 4) * 128
                    pg = ps_s.tile([128, 2, CH], f32, tag="s")
                    for h in range(2):
                        hb = h * 64
                        nc.tensor.matmul(
                            pg[:, h, off:CH],
                            qkvT[nk][hb:hb + 64, 1, offk:offk + 128],
                            qkvT[nq][hb:hb + 64, 0, off:CH],
                            start=True,
                            stop=(o is None),
                            tile_position=(hb, 0),
                            skip_group_check=True,
                        )
                    if o is not None:
                        # add -1e30 causal mask into the diagonal 128 cols of
                        # the score PSUM: += I^T @ M (mask is 0 past them)
                        for h in range(2):
                            nc.tensor.matmul(
                                pg[:, h, off:off + 128],
                                ident[:],
                                maskb_sb[:, :],
                                start=False,
                                stop=True,
                                skip_group_check=True,
                            )
                    pt_sb = ppool.tile([128, 2, CH], DT, tag="pT")
                    nc.scalar.activation(
                        out=pt_sb[:, :, off:CH],
                        in_=pg[:, :, off:CH],
                        func=mybir.ActivationFunctionType.Exp,
                        scale=0.125,
                    )
                    # pace independent dense work (proj of chunk n-1, qkv/V of
                    # chunk n+1) through the exp-bound loop so the PE never
                    # starves waiting for the ACT engine
                    target = (kt + 1) * len(dense_units) // ktmax
                    while emitted < target:
                        dense_units[emitted]()
                        emitted += 1
                    pending.append((kt, pt_sb, off))
                    if len(pending) > 2:
                        emit_av(*pending.pop(0))
                while emitted < len(dense_units):
                    dense_units[emitted]()
                    emitted += 1
                for p in pending:
                    emit_av(*p)
                # normalize both heads: copy den to partition 0 (regular DVE
                # copy handles the partition remap; the custom-DVE recip does
                # not), rec = 1/den; broadcast on gpsimd; yT = py * rec
                den2 = npool.tile([1, 2, CH], f32, tag="den")
                for h in range(2):
                    nc.vector.tensor_copy(out=den2[:, h, :], in_=py[h][64:65, :])
                rec2 = npool.tile([1, 2, CH], f32, tag="rec")
                nc.vector.reciprocal_approx_fast(out=rec2[:], in_=den2[:])
                bc = npool.tile([64, 2, CH], f32, tag="bc")
                nc.gpsimd.partition_broadcast(out_ap=bc[:], in_ap=rec2[:])
                for h in range(2):
                    nc.vector.tensor_mul(
                        out=yT[nq][h * 64:h * 64 + 64, :],
                        in0=py[h][0:64, :],
                        in1=bc[:, h, :],
                    )

            # engine streams execute IN ORDER; emission order is the PE
            # program. Chunk n's attention (exp-paced on ACT) is interleaved
            # at sub-chunk granularity with proj(n-1) and qkv/V of chunk n+1
            # so the PE stays busy (and HAM-warm) through the exp waits.
            # proj of chunk p is deferred into the chunk listed here: the
            # qc=3 chunks are exp(ACT)-paced with PE slack (chunk 7 has no
            # qkv fill at all), while early chunks are PE/DMA-bound — parking
            # proj work late fills the exp waits instead of adding wall.
            proj_in = {3: [0], 4: [1], 5: [2], 7: [3, 4, 5, 6]}
            qkv_m(0, 0)
            qkv_m(0, 1)
            for j in range(4):
                vdir_j(0, j)
            for n in range(NCH):
                b, qc = n // 4, n % 4
                units = []
                if n + 1 < NCH:
                    units += [lambda m=m, nn=n + 1: qkv_m(nn, m) for m in range(2)]
                    units += [lambda j=j, nn=n + 1: vdir_j(nn, j) for j in range(4)]
                for p in proj_in.get(n, []):
                    units += [lambda st=st, pp=p: proj_st(pp, st) for st in range(4)]
                attention_qc(b, qc, units)
            for st in range(4):
                proj_st(NCH - 1, st, last=True)

    nc.finalize()
    return nc


def make_core_inputs(x, W_attn, b_attn, W_proj):
    """Host-side sharding: slice per-core weights, transpose x, build mask."""
    np_dt = mybir.dt.np(DT)
    xT = np.ascontiguousarray(x.reshape(BS, E).T).astype(np_dt)  # [E, BS]

    # single additive causal mask for a diagonal 128x128 block:
    # valid iff j >= i (j = within-block q, i = k-in-tile)
    i = np.arange(128)[:, None]
    j = np.arange(128)[None, :]
    maskb = np.ascontiguousarray(
        np.where(j >= i, 0.0, MASK_VAL).astype(np_dt)
    )  # [128, 128]

    in_maps = []
    for c in range(NCORES):
        cols = slice(128 * c, 128 * (c + 1))
        wqkv = np.ascontiguousarray(
            np.concatenate(
                [W_attn[:, cols], W_attn[:, E:][:, cols], W_attn[:, 2 * E:][:, cols]],
                axis=1,
            )
        ).astype(np_dt)  # [E, 384]
        bq = np.ascontiguousarray(
            b_attn[cols].astype(np.float32)[:, None]
        )  # [128, 1] — q bias only (k bias is softmax-invariant, v bias is
        # folded into b_proj on the host)
        wp = np.ascontiguousarray(W_proj[128 * c:128 * (c + 1), :]).astype(np_dt)
        in_maps.append(
            {"xT": xT, "wqkv": wqkv, "bqkv": bq, "wproj": wp, "maskb": maskb}
        )
    return in_maps


_NC_CACHE = None


def kernel_run(inputs, trace=False):
    """Run the bass kernel; returns (full_output, BassKernelResults)."""
    global _NC_CACHE
    x = np.asarray(inputs["x"], dtype=np.float32)
    W_attn = np.asarray(inputs["W_attn"], dtype=np.float32)
    b_attn = np.asarray(inputs["b_attn"], dtype=np.float32)
    W_proj = np.asarray(inputs["W_proj"], dtype=np.float32)
    b_proj = np.asarray(inputs["b_proj"], dtype=np.float32)

    if _NC_CACHE is None:
        _NC_CACHE = build_nc()
    nc = _NC_CACHE

    in_maps = make_core_inputs(x, W_attn, b_attn, W_proj)
    res = run_bass_kernel_spmd(
        nc, in_maps, core_ids=list(range(NCORES)), trace=trace
    )
    acc = np.zeros((BS, E), dtype=np.float64)
    for r in res.results:
        acc += np.asarray(r["out"], dtype=np.float64)
    # b_proj_eff: v-bias contribution folds exactly through softmax + proj
    b_eff = b_proj.astype(np.float64) + b_attn[2 * E:].astype(
        np.float64
    ) @ W_proj.astype(np.float64)
    y = (acc + b_eff).astype(np.float32).reshape(B, S, E)
    return y, res


def kernel(**inputs):
    y, _ = kernel_run(inputs, trace=False)
    return y


if __name__ == "__main__":
    rng = np.random.default_rng(0)
    scale = 1.0 / np.sqrt(E)
    inputs = {
        "x": rng.standard_normal((B, S, E), dtype=np.float32),
        "W_attn": rng.standard_normal((E, 3 * E), dtype=np.float32) * scale,
        "b_attn": rng.standard_normal((3 * E,), dtype=np.float32) * 0.02,
        "W_proj": rng.standard_normal((E, E), dtype=np.float32) * scale,
        "b_proj": rng.standard_normal((E,), dtype=np.float32) * 0.02,
    }
    y = kernel(**inputs)
    print("kernel output", y.shape, y.dtype, float(np.abs(y).mean()))
